# revision 1
# baseline (speedup 1.0000x reference)
"""Fused LayerNorm + multi-head self-attention + out-projection for TRN2,
sharded over 8 NeuronCores as (batch x head-group): core c -> batch c//4,
heads [4*(c%4), 4*(c%4)+4).

Per-core pipeline (all matmuls bf16; ln_g folded into weights on host):
  phase 1: xn = LayerNorm(x[b]) token-major (per-partition scalars), xnT via
           DMA-xbar transpose (ring of 3 groups); qkT = W_qk^T xnT; V token-
           major. K/Q for heads 0-3 emitted first; leftovers (h2/h3 cols, V)
           injected into the first attention iterations' pacing slack.
  phase 2 (8 iterations n = qb*4+h; software-pipelined):
           per kt: S tile [128 k, 1024 q] = K_h^T Q_h (contract d=64, PE),
           paired with the PREVIOUS iteration's AV kt-pass (PE), then
           exp(SCALE*S) on ACT (PSUM->SBUF bf16 into an 18-slot ring),
           mask multiply (DVE, some kts on GPSIMD).
           AV reoriented token-major: o[q,d] accumulates over kt with
           stationary P-slices [128,128] and moving [V_h|1] [128,65];
           the ones-column gives the softmax denominator per-q ON PARTITIONS
           so normalize+evict is one per-partition tensor_scalar.
  phase 3 (per qb, spread across the next iteration's kt slots):
           O token-major -> DMA-xbar transpose -> O^T; out-proj per q-tile
           (PSUM shared with the S pool); bf16 out; host sums 4 partials.
"""

import numpy as np
import ml_dtypes
from contextlib import ExitStack

import concourse.bass as bass
import concourse.tile as tile
from concourse import mybir
from concourse.bass_utils import run_bass_kernel_spmd
import json as _json


def _split_waits(bir_json_bytes, max_waits=1):
    """This walrus build accepts only one sync-wait command per instruction;
    hoist extra Tile-emitted waits onto standalone EventSemaphore ops."""
    m = _json.loads(bir_json_bytes)
    n = 0
    for func in m["functions"]:
        for blk in func["blocks"]:
            out = []
            for inst in blk["instructions"]:
                si = inst.get("sync_info") or {}
                ow = si.get("on_wait") or []
                if len(ow) > max_waits:
                    for w in ow[:-max_waits]:
                        n += 1
                        out.append({
                            "engine": inst["engine"], "ins": [], "outs": [],
                            "name": f"WSPLIT-{n}",
                            "opcode": "EventSemaphore",
                            "sync_info": {"on_update": [], "on_wait": [w]},
                        })
                    si["on_wait"] = ow[-max_waits:]
                out.append(inst)
            blk["instructions"] = out
    return _json.dumps(m).encode()


F32 = mybir.dt.float32
BF16 = mybir.dt.bfloat16
AF = mybir.ActivationFunctionType

B, N, DIM = 2, 2048, 1024
HEADS, DH = 16, 64
HPC = 4                      # heads per core
SCALE = DH ** -0.5
LN_EPS = 1e-5
P = 128
NT = N // P                  # 16 token tiles
KD = DIM // P                # 8 contraction tiles over model dim
RS = 32                      # P-tile ring: two full iterations (no WAR coupling)
POOL_MASK_KT = ()   # Pool-engine masks serialize the exp chain; keep on DVE


def build_program(use_bias=False, ab=()):
    ab = set(ab)
    nc = bass.Bass()
    x_d = nc.dram_tensor("x", [N, DIM], BF16, kind="ExternalInput")
    keep_d = nc.dram_tensor("keep", [HPC, N, N], BF16, kind="ExternalInput")
    wqk_d = nc.dram_tensor("wqk", [DIM, 2 * HPC * DH], BF16, kind="ExternalInput")
    wv_d = nc.dram_tensor("wv", [DIM, HPC * DH], BF16, kind="ExternalInput")
    wo_d = nc.dram_tensor("wo", [HPC * DH, DIM], BF16, kind="ExternalInput")
    if use_bias:
        qkb_d = nc.dram_tensor("qkb", [2 * HPC * DH], F32, kind="ExternalInput")
        vb_d = nc.dram_tensor("vb", [1, HPC * DH], BF16, kind="ExternalInput")
    out_d = nc.dram_tensor("out", [N, DIM], BF16, kind="ExternalOutput")
    if "dbg" in ab:
        qkT_d = nc.dram_tensor("dbg_qkT", [P, 4, N], BF16, kind="ExternalOutput")
        v_d = nc.dram_tensor("dbg_v", [P, NT * HPC * DH], BF16, kind="ExternalOutput")
        otok_d = nc.dram_tensor("dbg_otok", [P, 2 * 8 * 2 * P], BF16, kind="ExternalOutput")

    with tile.TileContext(nc) as tc, ExitStack() as ctx:
        persist = ctx.enter_context(tc.tile_pool(name="persist", bufs=1))
        eps_t = persist.tile([P, 1], F32, tag="eps")
        nc.vector.memset(eps_t, LN_EPS)
        qkT = persist.tile([P, 4, N], BF16, tag="qkT")
              # m: 0=q(h01) 1=q(h23) 2=k(h01) 3=k(h23); partition=dh within pair
        v_all = persist.tile([P, NT, HPC, DH], BF16, tag="v_all")
        ones_t = persist.tile([P, 1], BF16, tag="ones_t")
        nc.vector.memset(ones_t, 1.0)
        o_sb = persist.tile([P, 2, N], BF16, tag="o_sb")   # O^T rows: [h01, h23]
        otok = persist.tile([P, 2, 8, 2 * P], BF16, tag="otok")
        wo_sb = persist.tile([P, 2, DIM], BF16, tag="wo")
        if use_bias:
            qkb_sb = persist.tile([P, 4], F32, tag="qkb")
            nc.sync.dma_start(out=qkb_sb, in_=qkb_d.rearrange("(t p) -> p t", p=P))
            vb_sb = persist.tile([1, 256], BF16, tag="vb")
            nc.sync.dma_start(out=vb_sb, in_=vb_d[:, :])
            ones1 = persist.tile([1, P], BF16, tag="ones1")
            nc.vector.memset(ones1, 1.0)

        keep_pool = ctx.enter_context(tc.tile_pool(name="keep", bufs=4))
        pb_pool = ctx.enter_context(tc.tile_pool(name="pbuf", bufs=1))
        st_pool = ctx.enter_context(tc.tile_pool(name="stats", bufs=6))
        rec_pool = ctx.enter_context(tc.tile_pool(name="rec", bufs=8))
        oev_pool = ctx.enter_context(tc.tile_pool(name="oev", bufs=4))
        ps_s = ctx.enter_context(tc.tile_pool(name="ps_s", bufs=2, space="PSUM"))
        ps_o8 = ctx.enter_context(tc.tile_pool(name="ps_o8", bufs=1, space="PSUM"))
        o8a = ps_o8.tile([P, 8, DH], F32, tag="o8a")
        o8b = ps_o8.tile([P, 8, DH], F32, tag="o8b")
        o8d_all = ps_o8.tile([P, 16, 1], F32, tag="o8d")
        o8s = [o8a, o8b]
        o8ds = [o8d_all[:, 0:8, :], o8d_all[:, 8:16, :]]
        ps_po = ctx.enter_context(tc.tile_pool(name="ps_po", bufs=1, space="PSUM"))

        pbuf = pb_pool.tile([P, RS, 1024], BF16, tag="pbuf")

        # phase-1-scoped pools (closed after the last QKV group)
        p1 = ExitStack()
        w1_pool = p1.enter_context(tc.tile_pool(name="w1", bufs=1))
        xin_pool = p1.enter_context(tc.tile_pool(name="xin", bufs=2))
        xn_pool = p1.enter_context(tc.tile_pool(name="xn", bufs=2))
        xnr_pool = p1.enter_context(tc.tile_pool(name="xnr", bufs=1))

        xnr = xnr_pool.tile([P, KD, 3, 512], BF16, tag="xnr")  # ring of 3 groups
        xgs = {}

        def emit_x(g):
            xg = xin_pool.tile([P, 4, DIM], BF16, tag="x")
            nc.sync.dma_start(
                out=xg, in_=x_d.rearrange("(g a p) d -> g p a d", g=4, a=4)[g])
            xgs[g] = xg

        keeps = {}

        def emit_keep(n, k4s):
            qb, h = n // HPC, n % HPC
            cs = slice(qb * 1024, (qb + 1) * 1024)
            for k4 in k4s:
                kp = keep_pool.tile([P, 4, 1024], BF16, tag="keep")
                if "nokeepdma" in ab:
                    nc.gpsimd.memset(kp, 1.0)
                else:
                    nc.sync.dma_start(
                        out=kp,
                        in_=keep_d[h, k4 * 512:(k4 + 1) * 512, cs]
                            .rearrange("(a p) q -> p a q", a=4))
                keeps[(n, k4)] = kp

        def emit_ln_group(g, vec_evict):
            rg = g % 3
            for a in range(4):
                tt = g * 4 + a
                xt = xgs[g][:, a, :]
                stats = st_pool.tile([P, 2, 6], F32, tag="bn")
                xt2 = xt.rearrange("p (s d) -> p s d", s=2)
                for s in range(2):
                    nc.vector.bn_stats(out=stats[:, s, :], in_=xt2[:, s, :])
                mv = st_pool.tile([P, 2], F32, tag="mv")
                nc.vector.bn_aggr(out=mv, in_=stats)
                std = st_pool.tile([P, 1], F32, tag="std")
                nc.scalar.activation(std, mv[:, 1:2], AF.Sqrt, bias=eps_t)
                rstd = st_pool.tile([P, 1], F32, tag="rstd")
                nc.vector.reciprocal(rstd, std)
                nmr = st_pool.tile([P, 1], F32, tag="nmr")
                nc.vector.tensor_mul(nmr, mv[:, 0:1], rstd)
                nc.vector.tensor_scalar_mul(nmr, nmr, -1.0)
                xn = xn_pool.tile([P, DIM], BF16, tag="xn")
                nc.vector.tensor_scalar(xn, xt, rstd, nmr,
                                        op0=mybir.AluOpType.mult,
                                        op1=mybir.AluOpType.add)
                nc.sync.dma_start_transpose(xnr[:, :, rg, a * P:(a + 1) * P], xn)

        def emit_qkv(g, m, vec_evict):
            rg = g % 3
            pqt = ps_s.tile([P, 1024], F32, tag="s")
            pq = pqt[:, 0:512]
            for k in range(KD):
                nc.tensor.matmul(
                    pq, wqk_sb[:, k, m * P:(m + 1) * P], xnr[:, k, rg, :],
                    start=(k == 0), stop=(k == KD - 1))
            dst = qkT[:, m, g * 512:(g + 1) * 512]
            eng = nc.vector if vec_evict else nc.scalar
            if use_bias:
                if vec_evict:
                    nc.vector.tensor_scalar_add(dst, pq, qkb_sb[:, m:m + 1])
                else:
                    nc.scalar.activation(dst, pq, AF.Identity,
                                         bias=qkb_sb[:, m:m + 1])
            else:
                if vec_evict:
                    nc.vector.tensor_copy(dst, pq)
                else:
                    nc.scalar.copy(dst, pq)

        def emit_v(g, a, vec_evict):
            rg = g % 3
            tt = g * 4 + a
            pvt = ps_s.tile([P, 1024], F32, tag="s")
            pv = pvt[:, 0:256]
            if use_bias:
                nc.tensor.matmul(pv, ones1, vb_sb, start=True, stop=False)
            for k in range(KD):
                nc.tensor.matmul(
                    pv, xnr[:, k, rg, a * P:(a + 1) * P], wv_sb[:, k, :],
                    start=(not use_bias and k == 0), stop=(k == KD - 1))
            dst = v_all[:, tt, :, :].rearrange("p h d -> p (h d)")
            if vec_evict:
                nc.vector.tensor_copy(dst, pv)
            else:
                nc.scalar.copy(dst, pv)

        def s_block(n, kt):
            qb, h = n // HPC, n % HPC
            qrow = (h % 2) * DH
            qm, km = h // 2, 2 + h // 2
            sp = ps_s.tile([P, 1024], F32, tag="s")
            for j in range(2):
                qs = qb * 1024 + j * 512
                nc.tensor.matmul(
                    sp[:, j * 512:(j + 1) * 512],
                    qkT[qrow:qrow + DH, km, kt * P:(kt + 1) * P],
                    qkT[qrow:qrow + DH, qm, qs:qs + 512],
                    start=True, stop=True)
            pslc = pbuf[:, (16 * n + kt) % RS, :]
            nc.scalar.activation(pslc, sp, AF.Exp, bias=0.0, scale=SCALE)
            kpx = keeps[(n, kt // 4)][:, kt % 4, :]
            if kt in POOL_MASK_KT:
                nc.gpsimd.tensor_mul(pslc, pslc, kpx)
            else:
                nc.vector.tensor_mul(pslc, pslc, kpx)

        def av_chain(n, qt, kts):
            h = n % HPC
            o8 = o8s[n % 2]
            o8d = o8ds[n % 2]
            for kt in kts:
                pslc = pbuf[:, (16 * n + kt) % RS, qt * P:(qt + 1) * P]
                nc.tensor.matmul(
                    o8[:, qt, :], pslc, v_all[:, kt, h, :],
                    start=(kt == 0), stop=(kt == NT - 1))
                nc.tensor.matmul(
                    o8d[:, qt, :], pslc, ones_t,
                    start=(kt == 0), stop=(kt == NT - 1))

        def av_evict(n):
            qb, h = n // HPC, n % HPC
            o8 = o8s[n % 2]
            o8d = o8ds[n % 2]
            for qt in range(8):
                rec = rec_pool.tile([P, 1], F32, tag="rec")
                nc.vector.reciprocal(rec, o8d[:, qt, :])
                nc.vector.tensor_scalar_mul(
                    otok[:, qb, qt, h * DH:(h + 1) * DH], o8[:, qt, :], rec)

        def tail_tr(qb, qt):
            cs = slice(qb * 1024 + qt * P, qb * 1024 + (qt + 1) * P)
            nc.sync.dma_start_transpose(o_sb[:, :, cs], otok[:, qb, qt, :])

        def tail_po(qb, qt, act_evict=False):
            cs = slice(qb * 1024 + qt * P, qb * 1024 + (qt + 1) * P)
            ot = oev_pool.tile([P, DIM], BF16, tag="ot")
            for nn2 in range(2):
                po = ps_po.tile([P, 512], F32, tag="po")
                for kg in range(2):
                    nc.tensor.matmul(
                        po,
                        o_sb[:, kg, cs],
                        wo_sb[:, kg, nn2 * 512:(nn2 + 1) * 512],
                        start=(kg == 0), stop=(kg == 1))
                dst = ot[:, nn2 * 512:(nn2 + 1) * 512]
                if act_evict and nn2 == 1:
                    nc.scalar.copy(dst, po)
                else:
                    nc.vector.tensor_copy(dst, po)
            nc.sync.dma_start(out=out_d[cs, :], in_=ot)

        # ------------------------- emission -------------------------------
        emit_x(0)
        wv_sb = w1_pool.tile([P, KD, 256], BF16, tag="wv")
        nc.sync.dma_start(out=wv_sb, in_=wv_d.rearrange("(k p) c -> p k c", p=P))
        wqk_sb = w1_pool.tile([P, KD, 512], BF16, tag="wqk")
        nc.sync.dma_start(out=wqk_sb, in_=wqk_d.rearrange("(k p) c -> p k c", p=P))
        emit_x(1)

        leftovers = []
        emit_ln_group(0, vec_evict=False)
        emit_keep(0, [0])
        emit_qkv(0, 2, vec_evict=False)
        emit_qkv(0, 0, vec_evict=True)
        emit_keep(0, [1])
        leftovers += [lambda g=0: emit_qkv(g, 3, False), lambda g=0: emit_qkv(g, 1, False)]
        leftovers += [lambda g=0, a=a: emit_v(g, a, False) for a in range(4)]
        emit_x(2)
        emit_ln_group(1, vec_evict=False)
        emit_keep(0, [2])
        emit_qkv(1, 0, vec_evict=True)
        emit_qkv(1, 2, vec_evict=False)
        emit_keep(0, [3])
        leftovers += [lambda g=1: emit_qkv(g, 3, False), lambda g=1: emit_qkv(g, 1, False)]
        leftovers += [lambda g=1, a=a: emit_v(g, a, False) for a in range(4)]
        emit_x(3)

        for kt in range(4):
            if kt == 2:
                emit_keep(1, [0])
            if leftovers:
                leftovers.pop(0)()
            s_block(0, kt)
        emit_ln_group(2, vec_evict=True)
        emit_qkv(2, 2, vec_evict=False)
        emit_qkv(2, 0, vec_evict=True)
        leftovers += [lambda g=2: emit_qkv(g, 3, False), lambda g=2: emit_qkv(g, 1, False)]
        leftovers += [lambda g=2, a=a: emit_v(g, a, False) for a in range(4)]
        for kt in range(4, 8):
            if kt == 6:
                emit_keep(1, [1])
            s_block(0, kt)
            if leftovers:
                leftovers.pop(0)()
        emit_ln_group(3, vec_evict=True)
        emit_qkv(3, 2, vec_evict=False)
        emit_qkv(3, 0, vec_evict=True)
        leftovers += [lambda g=3: emit_qkv(g, 3, False), lambda g=3: emit_qkv(g, 1, False)]
        leftovers += [lambda g=3, a=a: emit_v(g, a, False) for a in range(4)]
        for kt in range(8, 12):
            if kt == 10:
                emit_keep(1, [2])
            if leftovers:
                leftovers.pop(0)()
            s_block(0, kt)
        nc.sync.dma_start(out=wo_sb, in_=wo_d.rearrange("(k p) c -> p k c", p=P))

        for kt in range(12, 16):
            if kt == 14:
                emit_keep(1, [3])
            if leftovers:
                leftovers.pop(0)()
            s_block(0, kt)
            if kt % 2 == 1 and leftovers:
                leftovers.pop(0)()

        def av_evict_qt(n, qt):
            qb, h = n // HPC, n % HPC
            o8 = o8s[n % 2]
            o8d = o8ds[n % 2]
            rec = rec_pool.tile([P, 1], F32, tag="rec")
            nc.vector.reciprocal(rec, o8d[:, qt, :])
            nc.vector.tensor_scalar_mul(
                otok[:, qb, qt, h * DH:(h + 1) * DH], o8[:, qt, :], rec)

        # all remaining phase-1 leftovers must be emitted before av(0) reads V
        while leftovers:
            leftovers.pop(0)()

        for n in range(1, 8):
            for kt in range(NT):
                if n + 1 < 8 and kt % 2 == 0 and kt < 8:
                    emit_keep(n + 1, [kt // 2])
                if leftovers and kt % 2 == 0:
                    leftovers.pop(0)()
                if n in (5, 6) and kt % 4 == 0:
                    tail_tr(0, (n - 5) * 4 + kt // 4)
                s_block(n, kt)
                av_chain(n - 1, kt // 2,
                         range(0, 8) if kt % 2 == 0 else range(8, NT))
                if n in (5, 6) and kt % 4 == 3:
                    tail_po(0, (n - 5) * 4 + kt // 4)
            av_evict(n - 1)
            # release phase-1 pools once every group's QKV/V has been emitted
            if n == 4 and not leftovers:
                p1.close()

        for qt in range(8):
            av_chain(7, qt, range(NT))
            av_evict_qt(7, qt)
            tail_tr(1, qt)
            if qt >= 2:
                tail_po(1, qt - 2, act_evict=True)
        tail_po(1, 6, act_evict=True)
        tail_po(1, 7, act_evict=True)
        if "dbg" in ab:
            nc.sync.dma_start(out=qkT_d[:, :, :], in_=qkT)
            nc.sync.dma_start(out=v_d[:, :], in_=v_all.rearrange("p a b c -> p (a b c)"))
            nc.sync.dma_start(out=otok_d[:, :], in_=otok.rearrange("p a b c -> p (a b c)"))

    return nc


_NC_CACHE = {}


def _get_program(use_bias=False):
    key = ("nc", use_bias)
    if key not in _NC_CACHE:
        nc = build_program(use_bias=use_bias)
        data = _split_waits(nc.to_json_bytes())
        nc.to_json_bytes = lambda: data
        _NC_CACHE[key] = nc
    return _NC_CACHE[key]


def _shard_inputs(x, attn_mask, ln_g, ln_b, w_qkv, w_out):
    x = np.asarray(x, np.float32)
    attn_mask = np.asarray(attn_mask)
    ln_g = np.asarray(ln_g, np.float32)
    ln_b = np.asarray(ln_b, np.float32)
    w_qkv = np.asarray(w_qkv, np.float32)
    w_out = np.asarray(w_out, np.float32)

    use_bias = bool(np.any(ln_b != 0.0))
    wg = w_qkv * ln_g[:, None]
    in_maps = []
    for c in range(8):
        b, g = c // 4, c % 4
        hs = slice(g * HPC * DH, (g + 1) * HPC * DH)        # inner dims of group
        wq = wg[:, 0 * DIM:1 * DIM][:, hs]                  # [1024, 256]
        wk = wg[:, 1 * DIM:2 * DIM][:, hs]
        wv = wg[:, 2 * DIM:3 * DIM][:, hs]
        wqk = np.concatenate([wq, wk], axis=1)              # [1024, 512]
        keep = (~attn_mask[b, g * HPC:(g + 1) * HPC]).transpose(0, 2, 1)
        im = {
            "x": np.ascontiguousarray(x[b]).astype(ml_dtypes.bfloat16),
            "keep": np.ascontiguousarray(keep).astype(ml_dtypes.bfloat16),
            "wqk": np.ascontiguousarray(wqk).astype(ml_dtypes.bfloat16),
            "wv": np.ascontiguousarray(wv).astype(ml_dtypes.bfloat16),
            "wo": np.ascontiguousarray(w_out[hs, :]).astype(ml_dtypes.bfloat16),
        }
        if use_bias:
            bq = ln_b @ w_qkv[:, 0 * DIM:1 * DIM][:, hs]
            bk = ln_b @ w_qkv[:, 1 * DIM:2 * DIM][:, hs]
            bv = (ln_b @ w_qkv[:, 2 * DIM:3 * DIM][:, hs]).reshape(1, -1)
            im["qkb"] = np.concatenate([bq, bk]).astype(np.float32)
            im["vb"] = bv.astype(ml_dtypes.bfloat16)
        in_maps.append(im)
    return in_maps, use_bias


def kernel(x, attn_mask, ln_g, ln_b, w_qkv, w_out):
    in_maps, use_bias = _shard_inputs(x, attn_mask, ln_g, ln_b, w_qkv, w_out)
    nc = _get_program(use_bias)
    res = run_bass_kernel_spmd(nc, in_maps, list(range(8)))
    parts = [np.asarray(r["out"], dtype=np.float32) for r in res.results]
    out = np.stack([parts[0] + parts[1] + parts[2] + parts[3],
                    parts[4] + parts[5] + parts[6] + parts[7]])
    return out.astype(np.float32)



# revision 32
# speedup vs baseline: 1.0319x; 1.0319x over previous
"""Fused LayerNorm + multi-head self-attention + out-projection for TRN2,
sharded over 8 NeuronCores as (batch x head-group): core c -> batch c//4,
heads [4*(c%4), 4*(c%4)+4).

Schedule (v2): the main loop is ACT-exp-rate-limited (1038ns per [128,1024]
score tile, 128 tiles/core), so everything else is scheduled around that
cadence:
  - PE warmup spin at t=0 so the p-state ramp finishes before real matmuls.
  - Lead-in: x/w DMA first, LN g0 (DVE stats) -> xbar transpose -> QKV(h01)
    for g0/g1 with ACT evictions; first S blocks are j-split ([128,512])
    so exp starts before Q of g1 arrives.
  - AV runs at lag 2 (AV(n-2) during n, per-kt passes paired with that kt's
    slot of n-2), opening ~2 iterations of PE slack for the QKV/V leftovers;
    lag catches up at n=4 (av2+av3), n7 runs its own AV at lag 0.
  - Mask multiplies run on [128, 2, 1024] kt-pairs, DVE by default with a
    per-(n,pair) table offloading some pairs to the (otherwise idle) Pool.
  - O^T is built by PE transpose (identity matmul) instead of DMA-xbar,
    out-proj per q-tile, evictions balanced DVE/ACT; host sums 4 partials.
"""

import numpy as np
import ml_dtypes
from contextlib import ExitStack

import concourse.bass as bass
import concourse.tile as tile
from concourse import mybir
from concourse.bass_utils import run_bass_kernel_spmd
import json as _json


def _split_waits(bir_json_bytes, max_waits=1):
    """This walrus build accepts only one sync-wait command per instruction;
    hoist extra Tile-emitted waits onto standalone EventSemaphore ops."""
    m = _json.loads(bir_json_bytes)
    n = 0
    for func in m["functions"]:
        for blk in func["blocks"]:
            out = []
            for inst in blk["instructions"]:
                si = inst.get("sync_info") or {}
                ow = si.get("on_wait") or []
                if len(ow) > max_waits:
                    for w in ow[:-max_waits]:
                        n += 1
                        out.append({
                            "engine": inst["engine"], "ins": [], "outs": [],
                            "name": f"WSPLIT-{n}",
                            "opcode": "EventSemaphore",
                            "sync_info": {"on_update": [], "on_wait": [w]},
                        })
                    si["on_wait"] = ow[-max_waits:]
                out.append(inst)
            blk["instructions"] = out
    return _json.dumps(m).encode()


F32 = mybir.dt.float32
BF16 = mybir.dt.bfloat16
AF = mybir.ActivationFunctionType

B, N, DIM = 2, 2048, 1024
HEADS, DH = 16, 64
HPC = 4                      # heads per core
SCALE = DH ** -0.5
LN_EPS = 1e-5
P = 128
NT = N // P                  # 16 token tiles
KD = DIM // P                # 8 contraction tiles over model dim
RS = 32                      # P-tile ring: two full iterations (lag-2 AV)

N_WARM = 55                  # PE ramp warmup matmuls (N=512 each)

# mask engine per (n, pair): pairs kt (0,1),(2,3)..(14,15) -> 'd' DVE, 'p' Pool
MASK_ENG = {n: "dddddddd" for n in range(8)}


def build_program(use_bias=False, ab=()):
    ab = set(ab)
    nc = bass.Bass()
    x_d = nc.dram_tensor("x", [N, DIM], BF16, kind="ExternalInput")
    keep_d = nc.dram_tensor("keep", [HPC, N, N], BF16, kind="ExternalInput")
    wqk_d = nc.dram_tensor("wqk", [DIM, 2 * HPC * DH], BF16, kind="ExternalInput")
    wv_d = nc.dram_tensor("wv", [DIM, HPC * DH], BF16, kind="ExternalInput")
    wo_d = nc.dram_tensor("wo", [HPC * DH, DIM], BF16, kind="ExternalInput")
    id_d = nc.dram_tensor("ident", [P, P], BF16, kind="ExternalInput")
    if use_bias:
        qkb_d = nc.dram_tensor("qkb", [2 * HPC * DH], F32, kind="ExternalInput")
        vb_d = nc.dram_tensor("vb", [1, HPC * DH], BF16, kind="ExternalInput")
    out_d = nc.dram_tensor("out", [N, DIM], BF16, kind="ExternalOutput")
    if "dbg" in ab:
        qkT_d = nc.dram_tensor("dbg_qkT", [P, 4, N], BF16, kind="ExternalOutput")
        v_d = nc.dram_tensor("dbg_v", [P, NT * HPC * DH], BF16, kind="ExternalOutput")
        otok_d = nc.dram_tensor("dbg_otok", [P, 2 * 8 * 2 * P], BF16, kind="ExternalOutput")

    with tile.TileContext(nc) as tc, ExitStack() as ctx:
        persist = ctx.enter_context(tc.tile_pool(name="persist", bufs=1))
        eps_t = persist.tile([P, 1], F32, tag="eps")
        nc.vector.memset(eps_t, LN_EPS)
        qkT = persist.tile([P, 4, N], BF16, tag="qkT")
              # m: 0=q(h01) 1=q(h23) 2=k(h01) 3=k(h23); partition=dh within pair
        v_all = persist.tile([P, NT, HPC, DH], BF16, tag="v_all")
        ones_t = persist.tile([P, 1], BF16, tag="ones_t")
        nc.vector.memset(ones_t, 1.0)
        o_sb = persist.tile([P, 2, N], BF16, tag="o_sb")   # O^T rows: [h01, h23]
        otok = persist.tile([P, 2, 8, 2 * P], BF16, tag="otok")
        wo_sb = persist.tile([P, 2, DIM], BF16, tag="wo")
        id_sb = persist.tile([P, P], BF16, tag="ident")
        warm_sb = persist.tile([P, 512], BF16, tag="warm")
        if use_bias:
            qkb_sb = persist.tile([P, 4], F32, tag="qkb")
            nc.sync.dma_start(out=qkb_sb, in_=qkb_d.rearrange("(t p) -> p t", p=P))
            vb_sb = persist.tile([1, 256], BF16, tag="vb")
            nc.sync.dma_start(out=vb_sb, in_=vb_d[:, :])
            ones1 = persist.tile([1, P], BF16, tag="ones1")
            nc.vector.memset(ones1, 1.0)

        keep_pool = ctx.enter_context(tc.tile_pool(name="keep", bufs=2))
        pb_pool = ctx.enter_context(tc.tile_pool(name="pbuf", bufs=1))
        st_pool = ctx.enter_context(tc.tile_pool(name="stats", bufs=6))
        sq_pool = ctx.enter_context(tc.tile_pool(name="sqd", bufs=1))
        rec_pool = ctx.enter_context(tc.tile_pool(name="rec", bufs=8))
        oev_pool = ctx.enter_context(tc.tile_pool(name="oev", bufs=3))
        ps_s = ctx.enter_context(tc.tile_pool(name="ps_s", bufs=2, space="PSUM"))
        ps_o8 = ctx.enter_context(tc.tile_pool(name="ps_o8", bufs=1, space="PSUM"))
        o8a = ps_o8.tile([P, 8, DH], F32, tag="o8a")
        o8b = ps_o8.tile([P, 8, DH], F32, tag="o8b")
        o8s = [o8a, o8b]
        ps_aux = ctx.enter_context(tc.tile_pool(name="ps_aux", bufs=1, space="PSUM"))
        aux = ps_aux.tile([P, 512], F32, tag="aux")
        # one PSUM bank shared by the AV denominators and the O^T transpose
        # staging (disjoint byte ranges)
        o8ds = [aux[:, 0:8].rearrange("p (a o) -> p a o", o=1),
                aux[:, 8:16].rearrange("p (a o) -> p a o", o=1)]
        otr_ps = aux[:, 128:256].bitcast(BF16).rearrange("p (a b) -> p a b", a=2)
        ps_po = ctx.enter_context(tc.tile_pool(name="ps_po", bufs=1, space="PSUM"))

        pbuf = pb_pool.tile([P, RS, 1024], BF16, tag="pbuf")

        # phase-1-scoped pools (closed after the last QKV group)
        p1 = ExitStack()
        w1_pool = p1.enter_context(tc.tile_pool(name="w1", bufs=1))
        xin_pool = p1.enter_context(tc.tile_pool(name="xin", bufs=2))
        xn_pool = p1.enter_context(tc.tile_pool(name="xn", bufs=4))
        xnr_pool = p1.enter_context(tc.tile_pool(name="xnr", bufs=1))

        xnr = xnr_pool.tile([P, KD, 4, 512], BF16, tag="xnr")  # one slot per group
        xgs = {}

        def emit_x(g):
            xg = xin_pool.tile([P, 4, DIM], BF16, tag="x")
            nc.sync.dma_start(
                out=xg, in_=x_d.rearrange("(g a p) d -> g p a d", g=4, a=4)[g])
            xgs[g] = xg

        keeps = {}

        def emit_keep(n, k4s, split=False):
            qb, h = n // HPC, n % HPC
            cs = slice(qb * 1024, (qb + 1) * 1024)
            for k4 in k4s:
                kp = keep_pool.tile([P, 4, 1024], BF16, tag="keep")
                src = keep_d[h, k4 * 512:(k4 + 1) * 512, cs] \
                    .rearrange("(a p) q -> p a q", a=4)
                if split:
                    for a in range(4):
                        nc.sync.dma_start(out=kp[:, a, :], in_=src[:, a, :])
                else:
                    nc.sync.dma_start(out=kp, in_=src)
                keeps[(n, k4)] = kp

        def warm(k):
            # keep PE busy so the p-state ramp completes before real matmuls
            for _ in range(k):
                pw = ps_po.tile([P, 512], F32, tag="po")
                nc.tensor.matmul(pw, warm_sb[:, 0:P], warm_sb,
                                 start=True, stop=True)

        def emit_ln_tile(g, a, stats_eng="dve", tr="dma"):
            rg = g % 4
            xt = xgs[g][:, a, :]
            if stats_eng == "act":
                mv_m = st_pool.tile([P, 1], F32, tag="mvm")
                mv_v = st_pool.tile([P, 1], F32, tag="mvv")
                dump = sq_pool.tile([P, DIM], BF16, tag="sqd")
                sacc = st_pool.tile([P, 1], F32, tag="sacc")
                qacc = st_pool.tile([P, 1], F32, tag="qacc")
                nc.scalar.activation(dump, xt, AF.Identity, accum_out=sacc)
                nc.scalar.activation(dump, xt, AF.Square, accum_out=qacc)
                nc.vector.tensor_scalar_mul(mv_m, sacc, 1.0 / DIM)
                # var = E[x^2] - mu^2  (mu ~ 0 for this data; no cancellation)
                musq = st_pool.tile([P, 1], F32, tag="musq")
                nc.vector.tensor_mul(musq, mv_m, mv_m)
                nc.vector.tensor_scalar(mv_v, qacc, 1.0 / DIM, musq,
                                        op0=mybir.AluOpType.mult,
                                        op1=mybir.AluOpType.subtract)
            else:
                stats = st_pool.tile([P, 2, 6], F32, tag="bn")
                xt2 = xt.rearrange("p (s d) -> p s d", s=2)
                for s in range(2):
                    nc.vector.bn_stats(out=stats[:, s, :], in_=xt2[:, s, :])
                mv = st_pool.tile([P, 2], F32, tag="mv")
                nc.vector.bn_aggr(out=mv, in_=stats)
                mv_m, mv_v = mv[:, 0:1], mv[:, 1:2]
            std = st_pool.tile([P, 1], F32, tag="std")
            nc.scalar.activation(std, mv_v, AF.Sqrt, bias=eps_t)
            rstd = st_pool.tile([P, 1], F32, tag="rstd")
            nc.vector.reciprocal(rstd, std)
            nmr = st_pool.tile([P, 1], F32, tag="nmr")
            nc.vector.tensor_mul(nmr, mv_m, rstd)
            nc.vector.tensor_scalar_mul(nmr, nmr, -1.0)
            xn = xn_pool.tile([P, DIM], BF16, tag="xn")
            nc.vector.tensor_scalar(xn, xt, rstd, nmr,
                                    op0=mybir.AluOpType.mult,
                                    op1=mybir.AluOpType.add)
            if tr == "dma":
                nc.sync.dma_start_transpose(xnr[:, :, rg, a * P:(a + 1) * P], xn)
            else:
                # PE transpose via identity matmul; keeps the DMA queue free
                # during the lead-in. Evict split ACT/DVE to hide latency.
                stg = ps_po.tile([P, KD, P], BF16, tag="po", name="stg")
                for k in range(KD):
                    nc.tensor.matmul(stg[:, k, :], xn[:, k * P:(k + 1) * P],
                                     id_sb, is_transpose=True)
                dst = xnr[:, :, rg, a * P:(a + 1) * P]
                nc.scalar.copy(dst, stg)

        def emit_qkv(g, m, evict="dve", pool=None, ap=None):
            rg = g % 4
            if ap is not None:
                pq = ap
            elif pool is None:
                pqt = ps_s.tile([P, 1024], F32, tag="s", name="pqt")
                pq = pqt[:, 0:512]
            else:
                pq = pool.tile([P, 512], F32, tag="po", name="pqt")
            for k in range(KD):
                nc.tensor.matmul(
                    pq, wqk_sb[:, k, m * P:(m + 1) * P], xnr[:, k, rg, :],
                    start=(k == 0), stop=(k == KD - 1))
            dst = qkT[:, m, g * 512:(g + 1) * 512]
            if use_bias:
                if evict == "dve":
                    nc.vector.tensor_scalar_add(dst, pq, qkb_sb[:, m:m + 1])
                else:
                    nc.scalar.activation(dst, pq, AF.Identity,
                                         bias=qkb_sb[:, m:m + 1])
            else:
                if evict == "dve":
                    nc.vector.tensor_copy(dst, pq)
                else:
                    nc.scalar.copy(dst, pq)

        def emit_v(g, a, evict="dve", pool=None, ap=None):
            rg = g % 4
            tt = g * 4 + a
            if ap is not None:
                pv = ap[:, 0:256]
            elif pool is None:
                pvt = ps_s.tile([P, 1024], F32, tag="s", name="pvt")
                pv = pvt[:, 0:256]
            else:
                pv = pool.tile([P, 256], F32, tag="po", name="pvt")
            if use_bias:
                nc.tensor.matmul(pv, ones1, vb_sb, start=True, stop=False)
            for k in range(KD):
                nc.tensor.matmul(
                    pv, xnr[:, k, rg, a * P:(a + 1) * P], wv_sb[:, k, :],
                    start=(not use_bias and k == 0), stop=(k == KD - 1))
            dst = v_all[:, tt, :, :].rearrange("p h d -> p (h d)")
            if evict == "dve":
                nc.vector.tensor_copy(dst, pv)
            else:
                nc.scalar.copy(dst, pv)

        sps = {}

        def s_mm(n, kt, js=(0, 1)):
            # S matmuls for q-halves js; exp per half into pbuf (no mask yet)
            qb, h = n // HPC, n % HPC
            qrow = (h % 2) * DH
            qm, km = h // 2, 2 + h // 2
            if (n, kt) not in sps:
                sps[(n, kt)] = ps_s.tile([P, 1024], F32, tag="s", name="sp")
            sp = sps[(n, kt)]
            slot = (16 * n + kt) % RS
            for j in js:
                qs = qb * 1024 + j * 512
                nc.tensor.matmul(
                    sp[:, j * 512:(j + 1) * 512],
                    qkT[qrow:qrow + DH, km, kt * P:(kt + 1) * P],
                    qkT[qrow:qrow + DH, qm, qs:qs + 512],
                    start=True, stop=True)
            if js == (0, 1):
                nc.scalar.activation(pbuf[:, slot, :], sp, AF.Exp,
                                     bias=0.0, scale=SCALE)
                del sps[(n, kt)]
            else:
                j = js[0]
                nc.scalar.activation(
                    pbuf[:, slot, j * 512:(j + 1) * 512],
                    sp[:, j * 512:(j + 1) * 512], AF.Exp, bias=0.0, scale=SCALE)
                if j == 1:
                    del sps[(n, kt)]

        def mask_half(n, kt, j):
            slot = (16 * n + kt) % RS
            kpx = keeps[(n, kt // 4)][:, kt % 4, j * 512:(j + 1) * 512]
            dst = pbuf[:, slot, j * 512:(j + 1) * 512]
            nc.vector.tensor_mul(dst, dst, kpx)

        def mask_pair(n, kt):
            # kt even: mask kts (kt, kt+1) in one [128, 2, 1024] op
            slot = (16 * n + kt) % RS
            eng = MASK_ENG[n][kt // 2]
            kpx = keeps[(n, kt // 4)][:, kt % 4: kt % 4 + 2, :]
            dst = pbuf[:, slot:slot + 2, :]
            if eng == "p":
                nc.gpsimd.tensor_mul(dst, dst, kpx)
            else:
                nc.vector.tensor_mul(dst, dst, kpx)

        def av_pass(m, j):
            # j-th slot's accumulation step of AV iteration m (8 qt x 2 mm);
            # kt-grouped: legal for lag-2 (slot j read right before S(n, j)
            # overwrites it) and for lag-0 (right after mask(m, j)).
            h = m % HPC
            o8 = o8s[m % 2]
            o8d = o8ds[m % 2]
            slot = (16 * m + j) % RS
            for qt in range(8):
                pslc = pbuf[:, slot, qt * P:(qt + 1) * P]
                nc.tensor.matmul(
                    o8[:, qt, :], pslc, v_all[:, j, h, :],
                    start=(j == 0), stop=(j == NT - 1))
                nc.tensor.matmul(
                    o8d[:, qt, :], pslc, ones_t,
                    start=(j == 0), stop=(j == NT - 1))

        def av_burst(m, qt, evict_eng="dve"):
            # full 16-slot chain for one qt + immediate evict; lag-1 only
            # (m's slots are disjoint from the current n's S writes).
            h = m % HPC
            o8 = o8s[m % 2]
            o8d = o8ds[m % 2]
            for r in range(NT):
                slot = (16 * m + r) % RS
                pslc = pbuf[:, slot, qt * P:(qt + 1) * P]
                nc.tensor.matmul(
                    o8[:, qt, :], pslc, v_all[:, r, h, :],
                    start=(r == 0), stop=(r == NT - 1))
                nc.tensor.matmul(
                    o8d[:, qt, :], pslc, ones_t,
                    start=(r == 0), stop=(r == NT - 1))
            av_evict_qt(m, qt, eng=evict_eng)

        def av_evict_qt(m, qt, eng="dve"):
            qb, h = m // HPC, m % HPC
            o8 = o8s[m % 2]
            o8d = o8ds[m % 2]
            rec = rec_pool.tile([P, 1], F32, tag="rec")
            nc.vector.reciprocal(rec, o8d[:, qt, :])
            dst = otok[:, qb, qt, h * DH:(h + 1) * DH]
            if eng == "act":
                nc.scalar.activation(dst, o8[:, qt, :], AF.Copy, scale=rec)
            else:
                nc.vector.tensor_scalar_mul(dst, o8[:, qt, :], rec)

        def av_evict(m, eng="dve"):
            for qt in range(8):
                av_evict_qt(m, qt, eng)

        def tail_tr(qb, qt, evict="dve"):
            cs = slice(qb * 1024 + qt * P, qb * 1024 + (qt + 1) * P)
            nc.sync.dma_start_transpose(o_sb[:, :, cs], otok[:, qb, qt, :])

        def tail_po(qb, qt, evict="dve", po_aps=None):
            # po_aps: four [P,256] PSUM regions (idle o8 banks) -> fully
            # pipelined 256-wide chunks with DVE evicts; else one ps_s tile
            # per qt (tail only; attention is done) with split DVE/ACT evicts
            cs = slice(qb * 1024 + qt * P, qb * 1024 + (qt + 1) * P)
            ot = oev_pool.tile([P, DIM], BF16, tag="ot")
            if po_aps is not None:
                for c in range(4):
                    po = po_aps[c]
                    for kg in range(2):
                        nc.tensor.matmul(
                            po,
                            o_sb[:, kg, cs],
                            wo_sb[:, kg, c * 256:(c + 1) * 256],
                            start=(kg == 0), stop=(kg == 1))
                    nc.vector.tensor_copy(ot[:, c * 256:(c + 1) * 256], po)
            else:
                pot = ps_s.tile([P, 1024], F32, tag="s", name="pot")
                for nn2 in range(2):
                    po = pot[:, nn2 * 512:(nn2 + 1) * 512]
                    for kg in range(2):
                        nc.tensor.matmul(
                            po,
                            o_sb[:, kg, cs],
                            wo_sb[:, kg, nn2 * 512:(nn2 + 1) * 512],
                            start=(kg == 0), stop=(kg == 1))
                    dst = ot[:, nn2 * 512:(nn2 + 1) * 512]
                    nc.vector.tensor_copy(dst[:, 0:256], po[:, 0:256])
                    nc.scalar.copy(dst[:, 256:512], po[:, 256:512])
            nc.sync.dma_start(out=out_d[cs, :], in_=ot)

        # ------------------------- emission -------------------------------
        # DMA queue order is emission order (single SP HWDGE queue); the
        # lead-in sequence below is tuned so transposes and the first mask
        # slices land before their consumers.
        nc.gpsimd.memset(warm_sb, 0.0)
        emit_x(0)
        wqk_sb = w1_pool.tile([P, KD, 512], BF16, tag="wqk")
        wqk_src = wqk_d.rearrange("(k p) (m c) -> m p k c", p=P, c=P)
        for m in (2, 0):
            nc.sync.dma_start(out=wqk_sb[:, :, m * P:(m + 1) * P], in_=wqk_src[m])
        emit_keep(0, [0], split=True)
        emit_x(1)
        nc.sync.dma_start(out=id_sb, in_=id_d[:, :])
        warm(N_WARM)

        for a in range(4):
            emit_ln_tile(0, a, stats_eng="dve", tr="dma")
        emit_qkv(0, 2, evict="act")
        emit_qkv(0, 0, evict="act")
        # early attention: j0 halves of kt0/kt1 (q tokens 0-511, K from g0);
        # g1 QKV goes through the ps_po bank so the held j-split S tiles keep
        # contiguous ps_s pool lifetimes (else the pool gate deadlocks)
        for kt in range(2):
            s_mm(0, kt, js=(0,))
            mask_half(0, kt, 0)
        emit_x(2)
        for a in range(4):
            emit_ln_tile(1, a, stats_eng="dve", tr="dma")
        emit_x(3)
        # LN g2/g3 as early as DVE allows; their transposes ride the DMA
        # queue ahead of the later keeps
        for a in range(4):
            emit_ln_tile(2, a, stats_eng="dve", tr="dma")
        emit_keep(0, [1], split=True)
        emit_qkv(1, 0, evict="act", pool=ps_po)
        emit_qkv(1, 2, evict="dve", pool=ps_po)
        for kt in range(2):
            s_mm(0, kt, js=(1,))
            mask_half(0, kt, 1)
        for a in range(4):
            emit_ln_tile(3, a, stats_eng="dve", tr="dma")
        for kt in range(2, 4):
            s_mm(0, kt)
            if kt % 2 == 1:
                mask_pair(0, kt - 1)
        emit_keep(0, [2])
        for m in (1, 3):
            nc.sync.dma_start(out=wqk_sb[:, :, m * P:(m + 1) * P], in_=wqk_src[m])
        wv_sb = w1_pool.tile([P, KD, 256], BF16, tag="wv")
        nc.sync.dma_start(out=wv_sb, in_=wv_d.rearrange("(k p) c -> p k c", p=P))
        emit_qkv(2, 2, evict="dve", pool=ps_po)

        leftovers = []
        pop_i0 = [0]

        def L(fn):
            leftovers.append(fn)

        def pop_leftover():
            if leftovers:
                fn = leftovers.pop(0)
                if pop_i0[0] % 2 == 1:
                    fn(ap=o8a.rearrange("p a b -> p (a b)")[:, 0:512])
                else:
                    fn()
                pop_i0[0] += 1

        # phase-1 leftovers for g0/g1 (xnr slot rg0 must free before the g3
        # transposes; V needed once AV(0) starts in n2)
        L(lambda **kw: emit_qkv(0, 1, "dve", pool=ps_po, **kw))
        L(lambda **kw: emit_qkv(0, 3, "dve", pool=ps_po, **kw))
        for a in range(4):
            L(lambda a=a, **kw: emit_v(0, a, "dve", pool=ps_po, **kw))
        L(lambda **kw: emit_qkv(1, 1, "dve", pool=ps_po, **kw))
        L(lambda **kw: emit_qkv(1, 3, "dve", pool=ps_po, **kw))

        for kt in range(4, 8):
            pop_leftover()
            s_mm(0, kt)
            if kt % 2 == 1:
                mask_pair(0, kt - 1)
        emit_qkv(3, 2, evict="dve", pool=ps_po)
        nc.sync.dma_start(out=wo_sb, in_=wo_d.rearrange("(k p) c -> p k c", p=P))
        for kt in range(8, 16):
            pop_leftover()
            s_mm(0, kt)
            if kt % 2 == 1:
                mask_pair(0, kt - 1)
            if kt == 8:
                emit_keep(0, [3])
            if kt == 12:
                emit_keep(1, [0])

        # remaining phase-1 leftovers in deadline order: V-g1 + K(h23)-g2 by
        # n2-kt4/8, V-g2 by n2-kt8, K(h23)-g3 + V-g3 by n2-kt12, Q(h01)-g2/g3
        # by n4, Q(h23)-g2/g3 by n6
        for a in range(4):
            L(lambda a=a, **kw: emit_v(1, a, "dve", pool=ps_po, **kw))
        L(lambda **kw: emit_qkv(2, 3, "dve", pool=ps_po, **kw))
        for a in range(4):
            L(lambda a=a, **kw: emit_v(2, a, "dve", pool=ps_po, **kw))
        L(lambda **kw: emit_qkv(3, 3, "dve", pool=ps_po, **kw))
        for a in range(4):
            L(lambda a=a, **kw: emit_v(3, a, "dve", pool=ps_po, **kw))
        L(lambda **kw: emit_qkv(2, 0, "dve", pool=ps_po, **kw))
        L(lambda **kw: emit_qkv(3, 0, "dve", pool=ps_po, **kw))
        L(lambda **kw: emit_qkv(2, 1, "dve", pool=ps_po, **kw))
        L(lambda **kw: emit_qkv(3, 1, "dve", pool=ps_po, **kw))

        POPS = {1: (0, 1, 2, 3, 4, 6, 8, 10, 12, 14),
                2: (0, 1, 2, 3, 8, 12), 3: (0, 8)}
        # AV schedule: every stream is qt-contiguous bursts (interleaving
        # accumulation groups within one PSUM bank corrupts them). 'bunch' =
        # all 8 bursts at kt0 (lag-2: every read must precede the first S
        # overwrite of this n); 'burst' = one per odd kt (lag-1 safe).
        AVS = {2: ((0, "bunch"),), 3: ((1, "bunch"), (2, "burst")),
               4: ((3, "burst"),), 5: ((4, "burst"),), 6: ((5, "burst"),),
               7: ((6, "burst"),)}
        # pop psum regions: alternate ps_po with an idle o8 bank so leftover
        # projections double-buffer (o8a free until av(0)@n2, o8b till n3)
        o8a_ap = o8a.rearrange("p a b -> p (a b)")
        o8b_ap = o8b.rearrange("p a b -> p (a b)")
        pop_aux = {0: (o8a_ap,), 1: (o8a_ap, o8b_ap), 2: (o8b_ap,)}
        pop_i = [0]

        def pop_leftover2(n):
            if not leftovers:
                return
            fn = leftovers.pop(0)
            auxs = pop_aux.get(n, ())
            k = pop_i[0] % (len(auxs) + 1)
            if k == 0:
                fn(ap=None)
            else:
                fn(ap=auxs[k - 1][:, 0:512])
            pop_i[0] += 1

        for n in range(1, 8):
            avs = AVS.get(n, ())
            pops = POPS.get(n, ())
            for kt in range(NT):
                if kt in pops:
                    pop_leftover2(n)
                if kt == 0:
                    # bunch AFTER the remaining V pops: every V eviction must
                    # be emitted before these bursts read v_all
                    if n == 2:
                        for _ in range(3):
                            pop_leftover2(n)
                    for (m, mode) in avs:
                        if mode == "bunch":
                            for qt in range(8):
                                av_burst(m, qt, evict_eng="dve")
                s_mm(n, kt)
                if kt % 2 == 1:
                    mask_pair(n, kt - 1)
                    for (m, mode) in avs:
                        if mode == "burst":
                            av_burst(m, kt // 2, evict_eng="dve")
                    # qb0 O^T transpose once evicts 0-3 are in (n4), then
                    # out-proj chunks through the idle o8 banks (n5/n6)
                    if n == 4:
                        tail_tr(0, kt // 2, evict="dve")
                    # qb0 out-proj: 4 qts in n5 on the idle o8b bank, 4 in n6
                    # on o8a (av(4)/av(5) vacate them in turn)
                    if n == 5 and kt < 8:
                        aps = [o8b_ap[:, 0:256], o8b_ap[:, 256:512],
                               o8b_ap[:, 0:256], o8b_ap[:, 256:512]]
                        tail_po(0, kt // 2, po_aps=aps)
                    if n == 6 and kt < 8:
                        aps = [o8a_ap[:, 0:256], o8a_ap[:, 256:512],
                               o8a_ap[:, 0:256], o8a_ap[:, 256:512]]
                        tail_po(0, 4 + kt // 2, po_aps=aps)
                # keep prefetch (bufs=2): each emission must follow the
                # LAST emitted mask read of the buffer it recycles
                if kt == 0 and (n, 1) not in keeps:
                    emit_keep(n, [1])
                if kt == 4:
                    emit_keep(n, [2])
                if kt == 8:
                    emit_keep(n, [3])
                if kt == 12 and n + 1 < 8:
                    emit_keep(n + 1, [0])
        p1.close()

        # tail: evict av(7) (av(6) evicted in-loop), transpose, out-proj qb1
        # through the now-free ps_s pool (double-buffered)
        for qt in range(8):
            av_burst(7, qt, evict_eng="dve" if qt % 2 else "act")
            tail_tr(1, qt, evict="dve" if qt % 2 else "act")
            tail_po(1, qt)
        if "dbg" in ab:
            nc.sync.dma_start(out=qkT_d[:, :, :], in_=qkT)
            nc.sync.dma_start(out=v_d[:, :], in_=v_all.rearrange("p a b c -> p (a b c)"))
            nc.sync.dma_start(out=otok_d[:, :], in_=otok.rearrange("p a b c -> p (a b c)"))

    return nc


_NC_CACHE = {}


def _get_program(use_bias=False):
    key = ("nc", use_bias)
    if key not in _NC_CACHE:
        nc = build_program(use_bias=use_bias)
        data = _split_waits(nc.to_json_bytes())
        nc.to_json_bytes = lambda: data
        _NC_CACHE[key] = nc
    return _NC_CACHE[key]


def _shard_inputs(x, attn_mask, ln_g, ln_b, w_qkv, w_out):
    x = np.asarray(x, np.float32)
    attn_mask = np.asarray(attn_mask)
    ln_g = np.asarray(ln_g, np.float32)
    ln_b = np.asarray(ln_b, np.float32)
    w_qkv = np.asarray(w_qkv, np.float32)
    w_out = np.asarray(w_out, np.float32)

    use_bias = bool(np.any(ln_b != 0.0))
    wg = w_qkv * ln_g[:, None]
    ident = np.eye(P, dtype=np.float32)
    in_maps = []
    for c in range(8):
        b, g = c // 4, c % 4
        hs = slice(g * HPC * DH, (g + 1) * HPC * DH)        # inner dims of group
        wq = wg[:, 0 * DIM:1 * DIM][:, hs]                  # [1024, 256]
        wk = wg[:, 1 * DIM:2 * DIM][:, hs]
        wv = wg[:, 2 * DIM:3 * DIM][:, hs]
        wqk = np.concatenate([wq, wk], axis=1)              # [1024, 512]
        keep = (~attn_mask[b, g * HPC:(g + 1) * HPC]).transpose(0, 2, 1)
        im = {
            "x": np.ascontiguousarray(x[b]).astype(ml_dtypes.bfloat16),
            "keep": np.ascontiguousarray(keep).astype(ml_dtypes.bfloat16),
            "wqk": np.ascontiguousarray(wqk).astype(ml_dtypes.bfloat16),
            "wv": np.ascontiguousarray(wv).astype(ml_dtypes.bfloat16),
            "wo": np.ascontiguousarray(w_out[hs, :]).astype(ml_dtypes.bfloat16),
            "ident": ident.astype(ml_dtypes.bfloat16),
        }
        if use_bias:
            bq = ln_b @ w_qkv[:, 0 * DIM:1 * DIM][:, hs]
            bk = ln_b @ w_qkv[:, 1 * DIM:2 * DIM][:, hs]
            bv = (ln_b @ w_qkv[:, 2 * DIM:3 * DIM][:, hs]).reshape(1, -1)
            im["qkb"] = np.concatenate([bq, bk]).astype(np.float32)
            im["vb"] = bv.astype(ml_dtypes.bfloat16)
        in_maps.append(im)
    return in_maps, use_bias


def kernel(x, attn_mask, ln_g, ln_b, w_qkv, w_out):
    in_maps, use_bias = _shard_inputs(x, attn_mask, ln_g, ln_b, w_qkv, w_out)
    nc = _get_program(use_bias)
    res = run_bass_kernel_spmd(nc, in_maps, list(range(8)))
    parts = [np.asarray(r["out"], dtype=np.float32) for r in res.results]
    out = np.stack([parts[0] + parts[1] + parts[2] + parts[3],
                    parts[4] + parts[5] + parts[6] + parts[7]])
    return out.astype(np.float32)
